# revision 1
# baseline (speedup 1.0000x reference)
"""Bahdanau-style attention kernel for Trainium2, data-parallel over batch.

Math (per (s, b)):
    pre[s,b,:]  = We @ enc[s,b,:] + Wh @ hidden[b,:] + attn_b      (H outputs)
    energies    = score_w . tanh(pre)                               -> [S, B]
    out         = softmax over S of (energies masked to -1e12)      -> [B, 1, S]

Sharding: B=16 batches split 2-per-core over 8 NeuronCores; weights are
replicated; no collectives. Each core runs one identical Bass program on
its own input slice.

v7 vs the fp32r baseline (168us): everything bf16 (rel err ~2e-3 vs the
2e-2 gate, half the DMA bytes), no junk warm-up matmuls, and the
schedule is balanced around three measured facts: the PE's 216ns/matmul
pace, the DVE's ~1220ns per [128,1024] stt (dtype-independent), and the
~2.1us/chunk shared-HBM pace of the prologue.
  - Prologue on both HWDGE queues: weT k-chunks on sync, enc b0h0
    k-chunks on scalar (after tiny queue-warming DMAs), so pair k lands
    at the shared-HBM pace. b0h0's first 3 groups run k-major to match
    that pace; groups 3-7 s-major. All of enc stays resident
    (64KB/partition); the other three halves are single-trigger DMAs
    (fewer semaphores = shorter framework epilogue).
  - b0h0's groups drain to SBUF via ScalarE copies (frees PSUM banks
    without waiting on the bias, which needs Wh^T; Wh^T + bias/score
    constants are DMA'd on the scalar queue only after the first copy,
    keeping the prologue HBM pipe free).
  - The 8 deferred b0h0 drain chains are spread one per TWO groups
    across b0h1+b1h0, and their bias-adds run as gpsimd accumulate-DMAs
    (praw += bias_bc in SBUF) - the DVE can't carry more than ~1.4
    extra stt per group period (measured: batching them stalled the PE
    10us; gpsimd tensor ops are 2.8x slower than DVE - don't).
  - Inline drain chain: stt bias-add (DVE, reads PSUM), tanh (ScalarE,
    f32->bf16), score contraction as stt(bypass, mult) with accum_out
    written straight into the energies column (NOTE:
    tensor_tensor_reduce hangs TRN2 hardware here - do not use it).
    Masking is folded into one per-batch stt against a host-built 0/1
    mask AFTER exp (replaces 16 per-column mask adds).
  - bias_bc[b] = Wh@hidden_b + attn_b replicated to 128 partitions via
    PE broadcast (ones-row stationary). The hidden projection puts
    batch rows on partitions 0/32 (33-wide stationary) because ScalarE
    only reads PSUM at 32-aligned partitions.
  - softmax tail per batch: exp, masked stt with fused row-sums,
    total = ones matmul (cross-partition sum on the PE), reciprocal on
    DVE, PE broadcast of 1/sum, scale on ScalarE, 4x 32x32 DVE
    transposes, DMA out. The last group is split into two 512-wide
    half-groups to halve the final exposed drain chain.
    Max-subtraction is skipped (energies are O(1) bounded by
    |score_w|_1 so exp cannot overflow; masked exps are zeroed exactly,
    matching the reference's masked softmax).
"""

import sys

for _p in ("/opt/trn_rl_repo", "/opt/pypackages"):
    if _p not in sys.path:
        sys.path.append(_p)

import numpy as np
import ml_dtypes

from concourse import bacc, mybir, tile
from concourse.bass_utils import run_bass_kernel_spmd

H = 1024
S = 2048
B = 16
NCORES = 8
BL = B // NCORES  # local batches per core
P = 128
KT = H // P  # h_in chunks of 128
NT = S // P  # energies columns per batch = 16
NHALF = S // 2  # 1024 s positions per enc half
ST = NHALF // P  # s-tiles per half = 8

F32 = mybir.dt.float32
BF16 = mybir.dt.bfloat16
AF = mybir.ActivationFunctionType
OP = mybir.AluOpType

HALVES = [(0, 0), (0, 1), (1, 0), (1, 1)]  # (b, sh)


def _build_program():
    nc = bacc.Bacc("TRN2", target_bir_lowering=False, debug=False, num_devices=NCORES)

    encT = nc.dram_tensor("encT", [BL, H, S], BF16, kind="ExternalInput").ap()
    weT = nc.dram_tensor("weT", [H, H], BF16, kind="ExternalInput").ap()
    whT = nc.dram_tensor("whT", [H, H], BF16, kind="ExternalInput").ap()
    hid2 = nc.dram_tensor("hid2", [P, KT * 33], BF16, kind="ExternalInput").ap()
    battn_row = nc.dram_tensor("battn_row", [1, H], BF16, kind="ExternalInput").ap()
    score_bc = nc.dram_tensor("score_bc", [P, H], BF16, kind="ExternalInput").ap()
    maskbar = nc.dram_tensor("maskbar", [BL, P, NT], F32, kind="ExternalInput").ap()
    out = nc.dram_tensor("out", [BL, S], F32, kind="ExternalOutput").ap()

    with tile.TileContext(nc) as tc:
        with (
            tc.tile_pool(name="consts", bufs=1) as cpool,
            tc.tile_pool(name="weights", bufs=1) as wpool,
            tc.tile_pool(name="enc", bufs=1) as epool,
            tc.tile_pool(name="work", bufs=2) as ppool,
            tc.tile_pool(name="soft", bufs=1) as spool,
            tc.tile_pool(name="mm", bufs=3, space="PSUM") as mmpool,
            tc.tile_pool(name="aux", bufs=1, space="PSUM") as auxpool,
        ):
            # ---- gpsimd SWDGE queue: tiny constants (first needed ~40us,
            # so SWDGE fixed costs don't matter) --------------------------
            mask_sb = []
            for b in range(BL):
                m = cpool.tile([P, NT], F32, tag=f"maskbar{b}", name=f"maskbar{b}")
                nc.gpsimd.dma_start(m[:], maskbar[b])
                mask_sb.append(m)
            # hid2 column block k is 33 wide: col 0 = batch 0, col 32 =
            # batch 1 (rest zero), so the hidden-projection psum rows land
            # on partitions 0 and 32 - the 32-aligned bases ScalarE can
            # read PSUM from.
            hid_sb = cpool.tile([P, KT * 33], BF16, tag="hid2", name="hid_sb")
            nc.gpsimd.dma_start(hid_sb[:], hid2[:])

            # ---- prologue on BOTH HWDGE queues in parallel: weT k-chunks
            # on sync, enc b0h0 k-chunks on scalar. enc halves are single
            # big tiles [P, KT*H]; column block k holds enc chunk k -------
            ench = []
            for hi in range(len(HALVES)):
                ench.append(
                    epool.tile([P, KT * H], BF16, tag=f"ench{hi}", name=f"ench{hi}")
                )
            we_sb = []
            for k in range(KT):
                t = wpool.tile([P, H], BF16, tag=f"we{k}", name=f"we{k}")
                if k == 0:
                    # split the first chunks: DMA cold-start is roughly
                    # bandwidth-shaped, so a half chunk unblocks the first
                    # matmuls ~2us sooner. The enc k0 b-half (s-tiles 4-7)
                    # is deferred to after ench1-3: the k-major phase only
                    # touches s-tiles 0-3.
                    for hh in range(2):
                        nc.sync.dma_start(
                            t[:, hh * 512 : (hh + 1) * 512],
                            weT[0:P, hh * 512 : (hh + 1) * 512],
                        )
                    nc.scalar.dma_start(
                        ench[0][:, 0:512], encT[0, 0:P, 0:512]
                    )
                else:
                    nc.sync.dma_start(t[:], weT[k * P : (k + 1) * P, :])
                    nc.scalar.dma_start(
                        ench[0][:, k * H : (k + 1) * H],
                        encT[0, k * P : (k + 1) * P, 0:NHALF],
                    )
                we_sb.append(t)
                if k == 3:
                    nc.scalar.dma_start(
                        ench[0][:, 512:1024], encT[0, 0:P, 512:NHALF]
                    )
            for hi, (b, sh) in enumerate(HALVES[1:], start=1):
                for k in range(KT):
                    nc.sync.dma_start(
                        ench[hi][:, k * H : (k + 1) * H],
                        encT[b, k * P : (k + 1) * P, sh * NHALF : (sh + 1) * NHALF],
                    )

            # ---- tiny constants (DVE memsets; no DMA) -------------------
            ones_row_bf = cpool.tile([1, P], BF16, tag="ones_row_bf")
            nc.vector.memset(ones_row_bf[:], 1.0)
            ones_row_f = cpool.tile([1, P], F32, tag="ones_row_f")
            nc.vector.memset(ones_row_f[:], 1.0)
            ones_col_f = cpool.tile([P, 1], F32, tag="ones_col_f")
            nc.vector.memset(ones_col_f[:], 1.0)

            energies = []
            expd = []
            expm = []
            outsc = []
            outT = []
            colsum = []
            for b in range(BL):
                energies.append(
                    spool.tile([P, NT], F32, tag=f"energy{b}", name=f"energy{b}")
                )
                expd.append(
                    spool.tile([P, NT], F32, tag=f"expd{b}", name=f"expd{b}")
                )
                e = spool.tile([P, 32], F32, tag=f"expm{b}", name=f"expm{b}")
                nc.vector.memset(e[:, NT:32], 0.0)
                expm.append(e)
                o = spool.tile([P, 32], F32, tag=f"outsc{b}", name=f"outsc{b}")
                nc.vector.memset(o[:, NT:32], 0.0)
                outsc.append(o)
                outT.append(spool.tile([32, P], F32, tag=f"outT{b}", name=f"outT{b}"))
                colsum.append(
                    spool.tile([P, 1], F32, tag=f"colsum{b}", name=f"colsum{b}")
                )

            bias_bc = [
                cpool.tile([P, H], F32, tag=f"bias_bc{b}", name=f"bias_bc{b}")
                for b in range(BL)
            ]

            def group_mms(ps, hi, st, lo=0, nh=2):
                """Emit the accumulating matmuls of one psum group.

                A matmul can't cross a PSUM bank boundary, so each k issues
                512-wide matmuls (bank-halves lo..lo+nh) sharing the same
                stationary enc chunk.
                """
                for k in range(KT):
                    soff = k * H + st * P
                    for hh in range(lo, lo + nh):
                        nc.tensor.matmul(
                            ps[:, hh * 512 : (hh + 1) * 512],
                            lhsT=ench[hi][:, soff : soff + P],
                            rhs=we_sb[k][:, hh * 512 : (hh + 1) * 512],
                            start=(k == 0),
                            stop=(k == KT - 1),
                        )

            def drain(b, tix, src, deferred=False):
                """bias-add + tanh + score contraction into an energy col.

                Deferred (SBUF-sourced) drains do the bias-add as a gpsimd
                accumulate-DMA in place, keeping the DVE budget to one stt.
                """
                if deferred:
                    nc.gpsimd.dma_start(src[:], bias_bc[b][:], accum_op=OP.add)
                    pre = src
                else:
                    pre = ppool.tile([P, H], BF16, tag="pre", name=f"pre_{b}_{tix}")
                    nc.vector.scalar_tensor_tensor(
                        pre[:], src[:], 1.0, bias_bc[b][:], op0=OP.mult, op1=OP.add
                    )
                proj = ppool.tile([P, H], BF16, tag="proj", name=f"proj_{b}_{tix}")
                nc.scalar.activation(proj[:], pre[:], AF.Tanh)
                scr = ppool.tile([P, H], BF16, tag="scr", name=f"scr_{b}_{tix}")
                nc.vector.scalar_tensor_tensor(
                    scr[:],
                    proj[:],
                    0.0,
                    score_sb[:],
                    op0=OP.bypass,
                    op1=OP.mult,
                    accum_out=energies[b][:, tix : tix + 1],
                )

            def tail(b):
                """Masked softmax epilogue for one batch + store."""
                nc.scalar.activation(expd[b][:], energies[b][:], AF.Exp)
                nc.vector.scalar_tensor_tensor(
                    expm[b][:, 0:NT],
                    expd[b][:],
                    0.0,
                    mask_sb[b][:],
                    op0=OP.bypass,
                    op1=OP.mult,
                    accum_out=colsum[b][:],
                )
                tot = auxpool.tile([1, 1], F32, tag="aux", name=f"tot{b}")
                nc.tensor.matmul(
                    tot[:], lhsT=colsum[b][:], rhs=ones_col_f[:], start=True, stop=True
                )
                rec = spool.tile([1, 1], F32, tag=f"rec{b}", name=f"rec{b}")
                nc.vector.reciprocal(rec[:], tot[:])
                recb = auxpool.tile([P, 1], F32, tag="aux", name=f"recb{b}")
                nc.tensor.matmul(
                    recb[:], lhsT=ones_row_f[:], rhs=rec[:], start=True, stop=True
                )
                recs = spool.tile([P, 1], F32, tag=f"recs{b}", name=f"recs{b}")
                nc.scalar.copy(recs[:], recb[:])
                nc.scalar.mul(outsc[b][:, 0:NT], expm[b][:, 0:NT], recs[:])
                for q in range(4):
                    nc.vector.transpose(
                        outT[b][:, q * 32 : (q + 1) * 32],
                        outsc[b][q * 32 : (q + 1) * 32, :],
                    )
                nc.sync.dma_start(
                    out[b : b + 1, :].rearrange("o (t p) -> (o t) p", p=P),
                    outT[b][0:NT, :],
                )

            # ---- b0h0: k-major over the first 4 groups (8 matmuls per
            # k-level ~ the prologue's DMA chunk pace, so the PE never
            # dribbles; the 4th group borrows the idle aux PSUM slot),
            # then s-major for 4..7. Drains deferred via ScalarE copies
            # (bias needs Wh^T) -------------------------------------------
            # ---- HAM warm-up: ~3us of junk matmuls on memset data keep
            # the PE busy from ~7.5us while the first chunks land, so the
            # un-throttle (~4us of sustained work) happens before the real
            # stream needs full clock ------------------------------------
            junk_ps = auxpool.tile([P, P], F32, tag="aux", name="junk_ps")
            for _ in range(24):
                nc.tensor.matmul(
                    junk_ps[:],
                    lhsT=ones_row_bf[:],
                    rhs=ones_row_bf[:],
                    start=True,
                    stop=True,
                    skip_group_check=True,
                )

            praw = [
                ppool.tile([P, H], F32, tag=f"praw{st}", bufs=1, name=f"praw{st}")
                for st in range(ST)
            ]
            ps4 = [mmpool.tile([P, H], F32, tag="mm", name=f"ps4_{g}") for g in range(3)]
            ps4.append(auxpool.tile([P, H], F32, tag="aux", name="ps4_3"))
            # k=0 runs hh-blocked so its matmuls only wait on the half
            # chunks actually needed - a gap here would reset the HAM
            # un-throttle timer
            for hh in range(2):
                for g in range(4):
                    soff = g * P
                    nc.tensor.matmul(
                        ps4[g][:, hh * 512 : (hh + 1) * 512],
                        lhsT=ench[0][:, soff : soff + P],
                        rhs=we_sb[0][:, hh * 512 : (hh + 1) * 512],
                        start=True,
                        stop=False,
                    )
            for k in range(1, KT):
                for g in range(4):
                    soff = k * H + g * P
                    for hh in range(2):
                        nc.tensor.matmul(
                            ps4[g][:, hh * 512 : (hh + 1) * 512],
                            lhsT=ench[0][:, soff : soff + P],
                            rhs=we_sb[k][:, hh * 512 : (hh + 1) * 512],
                            start=False,
                            stop=(k == KT - 1),
                        )
            for g in range(4):
                # praw0 stays on ScalarE purely as the timing gate for the
                # Wh^T/constants trigger batch below; the rest go to the
                # (idle-during-b0h0) DVE so the trigger issue time doesn't
                # delay the copies that free the k-major PSUM slots.
                if g == 0:
                    nc.scalar.copy(praw[g][:], ps4[g][:])
                else:
                    nc.vector.tensor_scalar_add(praw[g][:], ps4[g][:], 0.0)
                if g == 0:
                    # Wh^T + the bias/score constants now: their
                    # scalar-queue DMAs are ordered after this copy,
                    # keeping the prologue HBM pipe free.
                    wh_sb = []
                    for k in range(KT):
                        t = wpool.tile([P, H], BF16, tag=f"wh{k}", name=f"wh{k}")
                        nc.scalar.dma_start(t[:], whT[k * P : (k + 1) * P, :])
                        wh_sb.append(t)
                    battn_sb = cpool.tile(
                        [1, H], BF16, tag="battn_row", name="battn_sb"
                    )
                    nc.scalar.dma_start(battn_sb[:], battn_row[:])
                    score_sb = cpool.tile(
                        [P, H], BF16, tag="score_bc", name="score_sb"
                    )
                    nc.scalar.dma_start(score_sb[:], score_bc[:])
            for st in range(4, ST):
                ps = mmpool.tile([P, H], F32, tag="mm", name=f"ps_00_{st}")
                group_mms(ps, 0, st)
                nc.vector.tensor_scalar_add(praw[st][:], ps[:], 0.0)

            # ---- hidden projection + bias broadcast ---------------------
            ps_h = auxpool.tile([33, H], F32, tag="aux", name="hidp")
            for k in range(KT):
                for hh in range(2):
                    nc.tensor.matmul(
                        ps_h[:, hh * 512 : (hh + 1) * 512],
                        lhsT=hid_sb[:, k * 33 : (k + 1) * 33],
                        rhs=wh_sb[k][:, hh * 512 : (hh + 1) * 512],
                        start=(k == 0),
                        stop=(k == KT - 1),
                    )
            # brow2[b] = Wh@hidden_b + attn_b as a single bf16 row; the
            # replicate-and-copy then needs no battn add, and the seeded
            # last groups can use the row directly as a PSUM bias seed.
            brow2 = []
            for b in range(BL):
                r = cpool.tile([1, H], BF16, tag=f"bias_row{b}", name=f"brow{b}")
                nc.scalar.copy(r[:], ps_h[b * 32 : b * 32 + 1, :])
                r2 = cpool.tile([1, H], BF16, tag=f"bias_row2{b}", name=f"brow2{b}")
                nc.vector.tensor_add(r2[:], r[:], battn_sb[:])
                brow2.append(r2)
            for b in range(BL):
                ps_bc = auxpool.tile([P, H], F32, tag="aux", name=f"bias_ps{b}")
                for hh in range(2):
                    nc.tensor.matmul(
                        ps_bc[:, hh * 512 : (hh + 1) * 512],
                        lhsT=ones_row_bf[:],
                        rhs=brow2[b][:, hh * 512 : (hh + 1) * 512],
                        start=True,
                        stop=True,
                    )
                nc.scalar.copy(bias_bc[b][:], ps_bc[:])

            # ---- b0h1 + b1h0 inline, deferred b0h0 drains one per TWO
            # groups (DVE budget: inline chain ~2.44us of a 3.46us group
            # period; a deferred score-stt fits only every other one) -----
            for st in range(ST):
                ps = mmpool.tile([P, H], F32, tag="mm", name=f"ps_01_{st}")
                group_mms(ps, 1, st)
                drain(0, ST + st, ps)
                if st % 2 == 0:
                    drain(0, st // 2, praw[st // 2], deferred=True)
            for st in range(ST):
                ps = mmpool.tile([P, H], F32, tag="mm", name=f"ps_10_{st}")
                group_mms(ps, 2, st)
                drain(1, st, ps)
                if st % 2 == 0:
                    drain(0, 4 + st // 2, praw[4 + st // 2], deferred=True)
            tail(0)

            # ---- b1h1 inline; the second-to-last group borrows the idle
            # aux PSUM slot (one extra period of drain-latency slack for
            # the slot rotation at the end). The last TWO groups seed
            # their PSUM with the bias row via two extra matmuls, so
            # their drains skip the DVE bias-stt entirely (tanh reads
            # PSUM directly) - that shortens the exposed final chains by
            # a full stt + cross-engine hop each --------------------------
            for st in range(ST):
                seeded = st >= ST - 2
                if st == ST - 2:
                    ps = auxpool.tile([P, H], F32, tag="aux", name=f"ps_11_{st}")
                else:
                    ps = mmpool.tile([P, H], F32, tag="mm", name=f"ps_11_{st}")
                if not seeded:
                    group_mms(ps, 3, st)
                    drain(1, ST + st, ps)
                else:
                    for hh in range(2):
                        nc.tensor.matmul(
                            ps[:, hh * 512 : (hh + 1) * 512],
                            lhsT=ones_row_bf[:],
                            rhs=brow2[1][:, hh * 512 : (hh + 1) * 512],
                            start=True,
                            stop=False,
                        )
                    for k in range(KT):
                        soff = k * H + st * P
                        for hh in range(2):
                            nc.tensor.matmul(
                                ps[:, hh * 512 : (hh + 1) * 512],
                                lhsT=ench[3][:, soff : soff + P],
                                rhs=we_sb[k][:, hh * 512 : (hh + 1) * 512],
                                start=False,
                                stop=(k == KT - 1),
                            )
                    tix = ST + st
                    proj = ppool.tile([P, H], BF16, tag="proj", name=f"proj_s{st}")
                    nc.scalar.activation(proj[:], ps[:], AF.Tanh)
                    scr = ppool.tile([P, H], BF16, tag="scr", name=f"scr_s{st}")
                    nc.vector.scalar_tensor_tensor(
                        scr[:],
                        proj[:],
                        0.0,
                        score_sb[:],
                        op0=OP.bypass,
                        op1=OP.mult,
                        accum_out=energies[1][:, tix : tix + 1],
                    )
            tail(1)

    nc.compile()
    return nc


_NC = None


def _get_program():
    global _NC
    if _NC is None:
        _NC = _build_program()
    return _NC


def make_in_maps(hidden, encoder_outputs, seq_mask, attn_w, attn_b, score_w):
    """Slice/relayout/quantize the full inputs into 8 per-core input maps."""
    hidden = np.asarray(hidden, dtype=np.float32)
    encoder_outputs = np.asarray(encoder_outputs, dtype=np.float32)
    seq_mask = np.asarray(seq_mask, dtype=np.int32)
    attn_w = np.asarray(attn_w, dtype=np.float32)
    attn_b = np.asarray(attn_b, dtype=np.float32)
    score_w = np.asarray(score_w, dtype=np.float32)

    bf = ml_dtypes.bfloat16
    weT = np.ascontiguousarray(attn_w[:, H:].T).astype(bf)  # [h_in, h_out]
    whT = np.ascontiguousarray(attn_w[:, :H].T).astype(bf)  # [h_in, h_out]
    battn_row = np.ascontiguousarray(attn_b[None, :]).astype(bf)
    score_bc = np.ascontiguousarray(
        np.broadcast_to(score_w[0][None, :], (P, H))
    ).astype(bf)
    encT = encoder_outputs.transpose(1, 2, 0)  # [B, H, S]
    hidT = hidden[0].T  # [H, B]
    # maskbar[b, p, t] = 1 - seq_mask[b, t*P + p]  (1 = keep, 0 = masked)
    maskbar = np.ascontiguousarray(
        (1.0 - seq_mask.astype(np.float32)).reshape(B, NT, P).transpose(0, 2, 1)
    )

    in_maps = []
    for c in range(NCORES):
        bsl = slice(c * BL, (c + 1) * BL)
        hid_kpb = hidT[:, bsl].reshape(KT, P, BL).transpose(1, 0, 2)  # [P, KT, BL]
        hid2 = np.zeros((P, KT, 33), dtype=np.float32)
        hid2[:, :, 0] = hid_kpb[:, :, 0]
        hid2[:, :, 32] = hid_kpb[:, :, 1]
        hid2 = np.ascontiguousarray(hid2.reshape(P, KT * 33)).astype(bf)
        in_maps.append(
            {
                "encT": np.ascontiguousarray(encT[bsl]).astype(bf),
                "weT": weT,
                "whT": whT,
                "hid2": hid2,
                "battn_row": battn_row,
                "score_bc": score_bc,
                "maskbar": np.ascontiguousarray(maskbar[bsl]),
            }
        )
    return in_maps


def gather_output(results):
    outs = np.concatenate([results[c]["out"] for c in range(NCORES)], axis=0)
    return np.ascontiguousarray(outs[:, None, :].astype(np.float32))


def kernel(hidden, encoder_outputs, seq_mask, attn_w, attn_b, score_w):
    nc = _get_program()
    in_maps = make_in_maps(
        hidden, encoder_outputs, seq_mask, attn_w, attn_b, score_w
    )
    last_err = None
    for _attempt in range(3):
        try:
            res = run_bass_kernel_spmd(nc, in_maps, list(range(NCORES)))
            return gather_output(res.results)
        except Exception as e:  # rare transient NRT device errors on first exec
            last_err = e
            import time as _time

            _time.sleep(2.0)
    raise last_err



# revision 2
# speedup vs baseline: 1.3283x; 1.3283x over previous
"""Bahdanau-style attention kernel for Trainium2, data-parallel over batch.

Math (per (s, b)):
    pre[s,b,:]  = We @ enc[s,b,:] + Wh @ hidden[b,:] + attn_b      (H outputs)
    energies    = score_w . tanh(pre)                               -> [S, B]
    out         = softmax over S of (energies masked)               -> [B, 1, S]

Sharding: B=16 batches split 2-per-core over 8 NeuronCores; weights are
replicated; no collectives.

v8: fp8(e4m3) DoubleRow main GEMM in the FLIPPED orientation (h_out on
PSUM partitions, We-pair stationary, enc-pair moving, K=256 per matmul).
  - Main matmul count halves vs bf16: 4 k-pairs x 2 ho x 4 sb x 8 ho-
    blocks... = 128 DR matmuls per batch at N=512.
  - We is pre-scaled by 4096 on the host (its raw values are subnormal
    in e4m3); the 1/4096 un-scale and the per-ho bias column
    (Wh@hidden_b + attn_b, computed f32 on host) fold into the ScalarE
    tanh activation (bias is per-partition in this orientation) - no
    DVE bias pass, no seed matmuls.
  - Score contraction is a PE matmul (score column stationary, proj
    moving) accumulated into an SBUF energies row by cheap DVE adds;
    the [1,512] score output reuses the just-freed main PSUM bank, so
    the whole schedule fits in 8 banks.
  - Mask folds into the FIRST energies accumulation as a host-built
    additive offset row (0 keep / -50 masked): exp then gives ~1e-22
    for masked slots, matching the reference's exact-0 to float
    precision.
  - Loop: per batch, ho-pair windows x (kp outer, 4 sb inner) so each
    DoubleRow stationary load amortizes over 4 matmuls; window W0 of
    batch 0 consumes k-pairs at the prologue DMA cadence (weP + enc
    halves interleaved across both HWDGE queues).
  - Tail per batch on a [1, 2048] energies row: exp with fused
    accum_out total, DVE reciprocal, scale split across ScalarE+DVE,
    one 8KB output DMA. No max-subtraction (energies are O(1)).
"""

import sys

for _p in ("/opt/trn_rl_repo", "/opt/pypackages"):
    if _p not in sys.path:
        sys.path.append(_p)

import numpy as np
import ml_dtypes

from concourse import bacc, mybir, tile
from concourse.bass_utils import run_bass_kernel_spmd

H = 1024
S = 2048
B = 16
NCORES = 8
BL = B // NCORES  # local batches per core
P = 128
KP = H // 256  # k-pairs of 256 = 4
HB = H // P  # ho blocks = 8
SB = S // 512  # s blocks of 512 = 4
WSCALE = 4096.0

F32 = mybir.dt.float32
BF16 = mybir.dt.bfloat16
FP8 = mybir.dt.float8e4
AF = mybir.ActivationFunctionType
OP = mybir.AluOpType
PM = mybir.MatmulPerfMode


def _build_program():
    nc = bacc.Bacc("TRN2", target_bir_lowering=False, debug=False, num_devices=NCORES)

    encP = nc.dram_tensor("encP", [BL, KP, P, 2 * S], FP8, kind="ExternalInput").ap()
    weP = nc.dram_tensor("weP", [KP, P, 2 * H], FP8, kind="ExternalInput").ap()
    biasc = nc.dram_tensor("biasc", [P, BL * HB], F32, kind="ExternalInput").ap()
    scorec = nc.dram_tensor("scorec", [P, HB], BF16, kind="ExternalInput").ap()
    moff = nc.dram_tensor("moff", [BL, S], F32, kind="ExternalInput").ap()
    out = nc.dram_tensor("out", [BL, S], F32, kind="ExternalOutput").ap()

    with tile.TileContext(nc) as tc:
        with (
            tc.tile_pool(name="consts", bufs=1) as cpool,
            tc.tile_pool(name="weights", bufs=1) as wpool,
            tc.tile_pool(name="enc", bufs=1) as epool,
            tc.tile_pool(name="work", bufs=3) as ppool,
            tc.tile_pool(name="soft", bufs=1) as spool,
            tc.tile_pool(name="mm", bufs=8, space="PSUM") as mmpool,
        ):
            # ---- tiny constants via gpsimd SWDGE (needed ~7us in) --------
            bias_sb = cpool.tile([P, BL * HB], F32, tag="biasc", name="bias_sb")
            nc.gpsimd.dma_start(bias_sb[:], biasc[:])
            score_sb = cpool.tile([P, HB], BF16, tag="scorec", name="score_sb")
            nc.gpsimd.dma_start(score_sb[:], scorec[:])
            moff_sb = []
            for b in range(BL):
                m = cpool.tile([1, S], F32, tag=f"moff{b}", name=f"moff{b}")
                nc.gpsimd.dma_start(m[:], moff[b : b + 1, :])
                moff_sb.append(m)

            # ---- prologue: interleave weP + enc-b0 halves on both HWDGE
            # queues so batch 0's k-pairs land at ~3-4us cadence ----------
            we_sb = [
                wpool.tile([P, 2, H], FP8, tag=f"we{kp}", name=f"we{kp}")
                for kp in range(KP)
            ]
            enc_sb = [
                [
                    epool.tile([P, 2, S], FP8, tag=f"enc{b}_{kp}", name=f"enc{b}_{kp}")
                    for kp in range(KP)
                ]
                for b in range(BL)
            ]

            def enc_half(b, kp, h):
                dst = enc_sb[b][kp][:, :, h * 1024 : (h + 1) * 1024]
                src = encP[b, kp].rearrange("p (i s) -> p i s", i=2)[
                    :, :, h * 1024 : (h + 1) * 1024
                ]
                return dst, src

            # sync queue: weP0, e0h1, e1h1, weP2, e2h1, e3h1, then b1 halves
            # scalar queue: e0h0, weP1, e1h0, e2h0, weP3, e3h0, then b1 halves
            nc.sync.dma_start(we_sb[0][:].rearrange("p i m -> p (i m)"), weP[0])
            d, s_ = enc_half(0, 0, 1)
            nc.sync.dma_start(d, s_)
            d, s_ = enc_half(0, 1, 1)
            nc.sync.dma_start(d, s_)
            nc.sync.dma_start(we_sb[2][:].rearrange("p i m -> p (i m)"), weP[2])
            d, s_ = enc_half(0, 2, 1)
            nc.sync.dma_start(d, s_)
            d, s_ = enc_half(0, 3, 1)
            nc.sync.dma_start(d, s_)

            d, s_ = enc_half(0, 0, 0)
            nc.scalar.dma_start(d, s_)
            nc.scalar.dma_start(we_sb[1][:].rearrange("p i m -> p (i m)"), weP[1])
            d, s_ = enc_half(0, 1, 0)
            nc.scalar.dma_start(d, s_)
            d, s_ = enc_half(0, 2, 0)
            nc.scalar.dma_start(d, s_)
            nc.scalar.dma_start(we_sb[3][:].rearrange("p i m -> p (i m)"), weP[3])
            d, s_ = enc_half(0, 3, 0)
            nc.scalar.dma_start(d, s_)

            for kp in range(KP):
                d, s_ = enc_half(1, kp, 1)
                nc.sync.dma_start(d, s_)
                d, s_ = enc_half(1, kp, 0)
                nc.scalar.dma_start(d, s_)

            # ---- DVE-memset constants -----------------------------------
            ones_row_bf = cpool.tile([1, P], BF16, tag="ones_row_bf")
            nc.vector.memset(ones_row_bf[:], 1.0)

            en_acc = [
                spool.tile([1, S], F32, tag=f"en_acc{b}", name=f"en_acc{b}")
                for b in range(BL)
            ]

            # ---- HAM warm-up: junk matmuls on memset data keep the PE
            # busy through the first DMA wait so the un-throttle fires by
            # the time real matmuls stream -------------------------------
            junk_ps = mmpool.tile([P, 512], F32, tag="mm", name="junk_ps")
            for _ in range(40):
                nc.tensor.matmul(
                    junk_ps[:, 0:P],
                    lhsT=ones_row_bf[:],
                    rhs=ones_row_bf[:],
                    start=True,
                    stop=True,
                    skip_group_check=True,
                )

            def tail(b):
                """Masked-softmax epilogue on the [1, S] energies row."""
                expd = spool.tile([1, S], F32, tag=f"expd{b}", name=f"expd{b}")
                tot = spool.tile([1, 1], F32, tag=f"tot{b}", name=f"tot{b}")
                nc.scalar.activation(
                    expd[:], en_acc[b][:], AF.Exp, accum_out=tot[:]
                )
                rec = spool.tile([1, 1], F32, tag=f"rec{b}", name=f"rec{b}")
                nc.vector.reciprocal(rec[:], tot[:])
                outrow = spool.tile([1, S], F32, tag=f"outrow{b}", name=f"outrow{b}")
                # split the scale across ScalarE and DVE to halve latency
                for q in range(SB):
                    sl = slice(q * 512, (q + 1) * 512)
                    if q % 2 == 0:
                        nc.scalar.mul(outrow[:, sl], expd[:, sl], rec[:])
                    else:
                        nc.vector.tensor_scalar(
                            outrow[:, sl], expd[:, sl], rec[:], None, op0=OP.mult
                        )
                nc.sync.dma_start(out[b : b + 1, :], outrow[:])

            # ---- main loops: per batch, ho-pair windows ------------------
            for b in range(BL):
                for W in range(HB // 2):
                    ps = [
                        [
                            mmpool.tile(
                                [P, 512], F32, tag="mm", name=f"ps{b}_{W}_{hi}_{sb}"
                            )
                            for sb in range(SB)
                        ]
                        for hi in range(2)
                    ]
                    for kp in range(KP):
                        for hi in range(2):
                            ho = 2 * W + hi
                            wsl = we_sb[kp][:, :, ho * P : (ho + 1) * P]
                            for sb in range(SB):
                                nc.tensor.matmul(
                                    ps[hi][sb][:],
                                    lhsT=wsl,
                                    rhs=enc_sb[b][kp][:, :, sb * 512 : (sb + 1) * 512],
                                    start=(kp == 0),
                                    stop=(kp == KP - 1),
                                    perf_mode=PM.DoubleRow,
                                )
                    for hi in range(2):
                        ho = 2 * W + hi
                        bcol = bias_sb[:, b * HB + ho : b * HB + ho + 1]
                        for sb in range(SB):
                            proj = ppool.tile(
                                [P, 512], BF16, tag="proj", name=f"proj{b}_{ho}_{sb}"
                            )
                            nc.scalar.activation(
                                proj[:],
                                ps[hi][sb][:],
                                AF.Tanh,
                                bias=bcol,
                                scale=1.0 / WSCALE,
                            )
                            en_ps = ps[hi][sb][0:1, :]
                            nc.tensor.matmul(
                                en_ps,
                                lhsT=score_sb[:, ho : ho + 1],
                                rhs=proj[:],
                                start=True,
                                stop=True,
                            )
                            acc = en_acc[b][:, sb * 512 : (sb + 1) * 512]
                            if ho == 0:
                                # first touch: seed with mask offset row
                                nc.vector.scalar_tensor_tensor(
                                    acc,
                                    en_ps,
                                    0.0,
                                    moff_sb[b][:, sb * 512 : (sb + 1) * 512],
                                    op0=OP.bypass,
                                    op1=OP.add,
                                )
                            else:
                                nc.vector.tensor_tensor(
                                    acc, acc, en_ps, op=OP.add
                                )
                tail(b)

    nc.compile()
    return nc


_NC = None


def _get_program():
    global _NC
    if _NC is None:
        _NC = _build_program()
    return _NC


def make_in_maps(hidden, encoder_outputs, seq_mask, attn_w, attn_b, score_w):
    """Slice/relayout/quantize the full inputs into 8 per-core input maps."""
    hidden = np.asarray(hidden, dtype=np.float32)
    encoder_outputs = np.asarray(encoder_outputs, dtype=np.float32)
    seq_mask = np.asarray(seq_mask, dtype=np.int32)
    attn_w = np.asarray(attn_w, dtype=np.float32)
    attn_b = np.asarray(attn_b, dtype=np.float32)
    score_w = np.asarray(score_w, dtype=np.float32)

    e4 = ml_dtypes.float8_e4m3fn
    bf = ml_dtypes.bfloat16

    # We^T [hin, ho] scaled into e4m3's normal range; pair layout
    # [KP, P, 2, ho]: hin = kp*256 + i*128 + p
    weT = attn_w[:, H:].T * WSCALE
    weP = np.ascontiguousarray(
        weT.reshape(KP, 2, P, H).transpose(0, 2, 1, 3).reshape(KP, P, 2 * H)
    ).astype(e4)

    # enc pair layout per batch: [B, KP, P, 2, S]
    encT = encoder_outputs.transpose(1, 2, 0)  # [B, H, S]
    encPf = encT.reshape(B, KP, 2, P, S).transpose(0, 1, 3, 2, 4).reshape(
        B, KP, P, 2 * S
    )
    encPq = np.ascontiguousarray(encPf).astype(e4)

    # bias columns: Wh @ hidden_b + attn_b, exact f32, [P, B*HB] with
    # col = b*HB + hb, row p = h_out hb*128+p
    hidb = hidden[0] @ attn_w[:, :H].T + attn_b  # [B, H]
    biasc_all = hidb.reshape(B, HB, P).transpose(2, 0, 1).reshape(P, B * HB)
    biasc_all = np.ascontiguousarray(biasc_all.astype(np.float32))

    scorec = np.ascontiguousarray(score_w[0].reshape(HB, P).T).astype(bf)

    # additive mask offsets: 0 keep, -50 masked (exp -> ~1e-22)
    moff_all = np.where(seq_mask != 0, np.float32(-50.0), np.float32(0.0))

    in_maps = []
    for c in range(NCORES):
        bsl = slice(c * BL, (c + 1) * BL)
        in_maps.append(
            {
                "encP": np.ascontiguousarray(encPq[bsl]),
                "weP": weP,
                "biasc": np.ascontiguousarray(
                    biasc_all[:, c * BL * HB : (c + 1) * BL * HB]
                ),
                "scorec": scorec,
                "moff": np.ascontiguousarray(moff_all[bsl]),
            }
        )
    return in_maps


def gather_output(results):
    outs = np.concatenate([results[c]["out"] for c in range(NCORES)], axis=0)
    return np.ascontiguousarray(outs[:, None, :].astype(np.float32))


def kernel(hidden, encoder_outputs, seq_mask, attn_w, attn_b, score_w):
    nc = _get_program()
    in_maps = make_in_maps(
        hidden, encoder_outputs, seq_mask, attn_w, attn_b, score_w
    )
    last_err = None
    for _attempt in range(3):
        try:
            res = run_bass_kernel_spmd(nc, in_maps, list(range(NCORES)))
            return gather_output(res.results)
        except Exception as e:  # rare transient NRT device errors on first exec
            last_err = e
            import time as _time

            _time.sleep(2.0)
    raise last_err


# revision 5
# speedup vs baseline: 1.3442x; 1.0120x over previous
"""Bahdanau-style attention kernel for Trainium2, data-parallel over batch.

Math (per (s, b)):
    pre[s,b,:]  = We @ enc[s,b,:] + Wh @ hidden[b,:] + attn_b      (H outputs)
    energies    = score_w . tanh(pre)                               -> [S, B]
    out         = softmax over S of (energies masked)               -> [B, 1, S]

Sharding: B=16 batches split 2-per-core over 8 NeuronCores; weights are
replicated; no collectives.

v9: fp8(e4m3) DoubleRow main GEMM in the FLIPPED orientation (h_out on
PSUM partitions, We-pair stationary, enc-pair moving, K=256 per matmul).
  - We pre-scaled by 4096 on host (raw values are subnormal in e4m3);
    the 1/4096 un-scale + per-ho bias column (Wh@hidden_b + attn_b,
    f32 on host) fold into the ScalarE tanh (bias is per-partition in
    this orientation) - no DVE bias pass, no seed matmuls.
  - Score contraction: PE matmuls (score column stationary, proj
    moving), 4 s-blocks CONCURRENT via col-tiling tile_position=(0,32j)
    writing partition 32j of each s-block's just-freed main PSUM bank;
    DVE accumulates [1,512] slices into an SBUF energies row. Score
    groups are DEFERRED into the next window's kp-loop (PE MM queue is
    strict FIFO - a score MM waiting on tanh would stall the stream).
  - Mask folds into the first energies accumulation as an additive
    offset row (0 keep / -50 masked).
  - Loop: per batch, ho-pair windows x (kp outer, 4 sb inner) so each
    DoubleRow stationary amortizes over 4 matmuls. The LAST window
    runs pair-major (kp-chain per bank) so drains pipeline into the
    tail instead of bunching at the end.
  - Cold start: memsets + ~48 junk matmuls emitted before everything
    (HAM un-throttle by the time real matmuls stream) and tiny
    queue-warming DMAs ahead of the big prologue chunks.
  - Tail per batch: per-sb exp with fused accum_out, combine, DVE
    reciprocal, scale split ScalarE/DVE, one 8KB out DMA.
"""

import sys

for _p in ("/opt/trn_rl_repo", "/opt/pypackages"):
    if _p not in sys.path:
        sys.path.append(_p)

import numpy as np
import ml_dtypes

from concourse import bacc, mybir, tile
from concourse.bass_utils import run_bass_kernel_spmd

H = 1024
S = 2048
B = 16
NCORES = 8
BL = B // NCORES  # local batches per core
P = 128
KP = H // 256  # k-pairs of 256 = 4
HB = H // P  # ho blocks = 8
SB = S // 512  # s blocks of 512 = 4
WSCALE = 4096.0

F32 = mybir.dt.float32
BF16 = mybir.dt.bfloat16
FP8 = mybir.dt.float8e4
AF = mybir.ActivationFunctionType
OP = mybir.AluOpType
PM = mybir.MatmulPerfMode


def _build_program():
    nc = bacc.Bacc("TRN2", target_bir_lowering=False, debug=False, num_devices=NCORES)

    encP = nc.dram_tensor("encP", [BL, KP, P, 2 * S], FP8, kind="ExternalInput").ap()
    weP = nc.dram_tensor("weP", [KP, P, 2 * H], FP8, kind="ExternalInput").ap()
    biasc = nc.dram_tensor("biasc", [P, BL * HB], F32, kind="ExternalInput").ap()
    scorec = nc.dram_tensor("scorec", [P, HB], BF16, kind="ExternalInput").ap()
    moff = nc.dram_tensor("moff", [BL, S], F32, kind="ExternalInput").ap()
    out = nc.dram_tensor("out", [BL, S], F32, kind="ExternalOutput").ap()

    with tile.TileContext(nc) as tc:
        with (
            tc.tile_pool(name="consts", bufs=1) as cpool,
            tc.tile_pool(name="weights", bufs=1) as wpool,
            tc.tile_pool(name="enc", bufs=1) as epool,
            tc.tile_pool(name="work", bufs=10) as ppool,
            tc.tile_pool(name="soft", bufs=1) as spool,
            tc.tile_pool(name="mm", bufs=8, space="PSUM") as mmpool,
        ):
            # ---- warm-up first: memset consts + junk matmuls so the PE
            # is busy (and HAM un-throttled) before real work arrives ----
            ones_row_bf = cpool.tile([1, P], BF16, tag="ones_row_bf")
            nc.vector.memset(ones_row_bf[:], 1.0)
            junk_ps = mmpool.tile([P, 512], F32, tag="mm", name="junk_ps")
            for _ in range(48):
                nc.tensor.matmul(
                    junk_ps[:, 0:P],
                    lhsT=ones_row_bf[:],
                    rhs=ones_row_bf[:],
                    start=True,
                    stop=True,
                    skip_group_check=True,
                )

            # ---- queue-warming tiny DMAs (HWDGE cold-start is several
            # us; a small transfer first gets the queue moving) ----------
            warm_a = cpool.tile([1, P], FP8, tag="warm_a", name="warm_a")
            warm_b = cpool.tile([1, P], FP8, tag="warm_b", name="warm_b")
            nc.sync.dma_start(warm_a[:], weP[0, 0:1, 0:P])
            nc.scalar.dma_start(warm_b[:], weP[0, 1:2, 0:P])

            # ---- tiny constants via gpsimd SWDGE (needed ~7us in) ------
            bias_sb = cpool.tile([P, BL * HB], F32, tag="biasc", name="bias_sb")
            nc.gpsimd.dma_start(bias_sb[:], biasc[:])
            score_sb = cpool.tile([P, HB], BF16, tag="scorec", name="score_sb")
            nc.gpsimd.dma_start(score_sb[:], scorec[:])
            moff_sb = []
            for b in range(BL):
                m = cpool.tile([1, S], F32, tag=f"moff{b}", name=f"moff{b}")
                nc.gpsimd.dma_start(m[:], moff[b : b + 1, :])
                moff_sb.append(m)

            # ---- prologue: interleave weP + enc-b0 halves on both HWDGE
            # queues so batch 0's k-pairs land at ~3-4us cadence ----------
            we_sb = [
                wpool.tile([P, 2, H], FP8, tag=f"we{kp}", name=f"we{kp}")
                for kp in range(KP)
            ]
            enc_sb = [
                [
                    epool.tile([P, 2, S], FP8, tag=f"enc{b}_{kp}", name=f"enc{b}_{kp}")
                    for kp in range(KP)
                ]
                for b in range(BL)
            ]

            def enc_half(b, kp, h):
                dst = enc_sb[b][kp][:, :, h * 1024 : (h + 1) * 1024]
                src = encP[b, kp].rearrange("p (i s) -> p i s", i=2)[
                    :, :, h * 1024 : (h + 1) * 1024
                ]
                return dst, src

            # sync queue: weP0, e0h1, e1h1, weP2, e2h1, e3h1
            # scalar queue: e0h0, weP1, e1h0, e2h0, weP3, e3h0
            nc.sync.dma_start(we_sb[0][:].rearrange("p i m -> p (i m)"), weP[0])
            d, s_ = enc_half(0, 0, 1)
            nc.sync.dma_start(d, s_)
            d, s_ = enc_half(0, 1, 1)
            nc.sync.dma_start(d, s_)
            nc.sync.dma_start(we_sb[2][:].rearrange("p i m -> p (i m)"), weP[2])
            d, s_ = enc_half(0, 2, 1)
            nc.sync.dma_start(d, s_)
            d, s_ = enc_half(0, 3, 1)
            nc.sync.dma_start(d, s_)

            d, s_ = enc_half(0, 0, 0)
            nc.scalar.dma_start(d, s_)
            nc.scalar.dma_start(we_sb[1][:].rearrange("p i m -> p (i m)"), weP[1])
            d, s_ = enc_half(0, 1, 0)
            nc.scalar.dma_start(d, s_)
            d, s_ = enc_half(0, 2, 0)
            nc.scalar.dma_start(d, s_)
            nc.scalar.dma_start(we_sb[3][:].rearrange("p i m -> p (i m)"), weP[3])
            d, s_ = enc_half(0, 3, 0)
            nc.scalar.dma_start(d, s_)

            # batch 1: full-chunk DMAs (2KB/partition lines), split queues
            for kp in range(KP):
                q = nc.sync if kp % 2 == 0 else nc.scalar
                q.dma_start(
                    enc_sb[1][kp][:].rearrange("p i s -> p (i s)"), encP[1, kp]
                )

            en_acc = [
                spool.tile([1, S], F32, tag=f"en_acc{b}", name=f"en_acc{b}")
                for b in range(BL)
            ]
            exp_sb = [
                spool.tile([1, S], F32, tag=f"expd{b}", name=f"expd{b}")
                for b in range(BL)
            ]
            tot_sb = [
                spool.tile([1, SB], F32, tag=f"tot{b}", name=f"tot{b}")
                for b in range(BL)
            ]

            def drains(b, ho, ps_row, last):
                """tanh all 4 s-blocks of one ho, then a col-tiled score
                matmul 4-pack + DVE energy accumulation.

                Returns a closure emitting the score pack + DVE adds; in
                steady state the caller defers it into the next window's
                kp-loop so PE-FIFO stalls on tanh never block DR matmuls.
                """
                bcol = bias_sb[:, b * HB + ho : b * HB + ho + 1]
                projs = []
                for sb in range(SB):
                    proj = ppool.tile(
                        [P, 512], BF16, tag="proj", name=f"proj{b}_{ho}_{sb}"
                    )
                    nc.scalar.activation(
                        proj[:], ps_row[sb][:], AF.Tanh, bias=bcol, scale=1.0 / WSCALE
                    )
                    projs.append(proj)

                def emit_scores():
                    for sb in range(SB):
                        en_ps = ps_row[sb][32 * sb : 32 * sb + 1, :]
                        nc.tensor.matmul(
                            en_ps,
                            lhsT=score_sb[:, ho : ho + 1],
                            rhs=projs[sb][:],
                            start=True,
                            stop=True,
                            tile_position=(0, 32 * sb),
                        )
                    for sb in range(SB):
                        en_ps = ps_row[sb][32 * sb : 32 * sb + 1, :]
                        acc = en_acc[b][:, sb * 512 : (sb + 1) * 512]
                        if ho == 0:
                            nc.vector.scalar_tensor_tensor(
                                acc,
                                en_ps,
                                0.0,
                                moff_sb[b][:, sb * 512 : (sb + 1) * 512],
                                op0=OP.bypass,
                                op1=OP.add,
                            )
                        else:
                            nc.vector.tensor_tensor(acc, acc, en_ps, op=OP.add)
                        if last:
                            # pipeline the tail: exp + per-sb total
                            nc.scalar.activation(
                                exp_sb[b][:, sb * 512 : (sb + 1) * 512],
                                acc,
                                AF.Exp,
                                accum_out=tot_sb[b][:, sb : sb + 1],
                            )

                return emit_scores

            def tail_combine(b):
                """Combine per-sb exp totals, reciprocal, scale, store."""
                t01 = spool.tile([1, 1], F32, tag=f"t01_{b}", name=f"t01_{b}")
                t23 = spool.tile([1, 1], F32, tag=f"t23_{b}", name=f"t23_{b}")
                tot = spool.tile([1, 1], F32, tag=f"tt_{b}", name=f"tt_{b}")
                nc.vector.tensor_tensor(
                    t01[:], tot_sb[b][:, 0:1], tot_sb[b][:, 1:2], op=OP.add
                )
                nc.vector.tensor_tensor(
                    t23[:], tot_sb[b][:, 2:3], tot_sb[b][:, 3:4], op=OP.add
                )
                nc.vector.tensor_tensor(tot[:], t01[:], t23[:], op=OP.add)
                rec = spool.tile([1, 1], F32, tag=f"rec{b}", name=f"rec{b}")
                nc.vector.reciprocal(rec[:], tot[:])
                outrow = spool.tile([1, S], F32, tag=f"outrow{b}", name=f"outrow{b}")
                for q in range(SB):
                    sl = slice(q * 512, (q + 1) * 512)
                    if q % 2 == 0:
                        nc.scalar.mul(outrow[:, sl], exp_sb[b][:, sl], rec[:])
                    else:
                        nc.vector.tensor_scalar(
                            outrow[:, sl], exp_sb[b][:, sl], rec[:], None, op0=OP.mult
                        )
                nc.sync.dma_start(out[b : b + 1, :], outrow[:])

            def tail_full(b):
                """Tail for a batch whose drains didn't pipeline exp."""
                for sb in range(SB):
                    sl = slice(sb * 512, (sb + 1) * 512)
                    nc.scalar.activation(
                        exp_sb[b][:, sl],
                        en_acc[b][:, sl],
                        AF.Exp,
                        accum_out=tot_sb[b][:, sb : sb + 1],
                    )
                tail_combine(b)

            # ---- main loops: per batch, ho-pair windows ------------------
            NW = HB // 2
            pending = []  # deferred score packs from the previous window
            pending_tail = None  # previous batch's tail
            for b in range(BL):
                for W in range(NW):
                    is_tail_w = (b == BL - 1) and (W == NW - 1)
                    ps = [
                        [
                            mmpool.tile(
                                [P, 512], F32, tag="mm", name=f"ps{b}_{W}_{hi}_{sb}"
                            )
                            for sb in range(SB)
                        ]
                        for hi in range(2)
                    ]
                    if not is_tail_w:
                        # kp-outer: each stationary amortizes over 4 MMs
                        for kp in range(KP):
                            for hi in range(2):
                                ho = 2 * W + hi
                                wsl = we_sb[kp][:, :, ho * P : (ho + 1) * P]
                                for sb in range(SB):
                                    nc.tensor.matmul(
                                        ps[hi][sb][:],
                                        lhsT=wsl,
                                        rhs=enc_sb[b][kp][
                                            :, :, sb * 512 : (sb + 1) * 512
                                        ],
                                        start=(kp == 0),
                                        stop=(kp == KP - 1),
                                        perf_mode=PM.DoubleRow,
                                    )
                            if kp == 1:
                                for fn in pending:
                                    fn()
                                pending = []
                                if pending_tail is not None:
                                    pending_tail()
                                    pending_tail = None
                        for hi in range(2):
                            ho = 2 * W + hi
                            em = drains(b, ho, ps[hi], last=False)
                            if hi == 0:
                                em()  # hi0's tanh is long done by kp3
                            else:
                                pending.append(em)
                    else:
                        # final window pair-major: each bank's kp-chain
                        # completes in turn so drains pipeline into the
                        # tail instead of bunching after the last matmul
                        emitted = {}
                        first = True
                        for hi in range(2):
                            ho = 2 * W + hi
                            for sb in range(SB):
                                for kp in range(KP):
                                    nc.tensor.matmul(
                                        ps[hi][sb][:],
                                        lhsT=we_sb[kp][:, :, ho * P : (ho + 1) * P],
                                        rhs=enc_sb[b][kp][
                                            :, :, sb * 512 : (sb + 1) * 512
                                        ],
                                        start=(kp == 0),
                                        stop=(kp == KP - 1),
                                        perf_mode=PM.DoubleRow,
                                    )
                                if first:
                                    for fn in pending:
                                        fn()
                                    pending = []
                                    first = False
                            emitted[hi] = drains(b, ho, ps[hi], last=(hi == 1))
                            emitted[hi]()
                        tail_combine(b)
                if b < BL - 1:
                    pending_tail = (lambda bb: (lambda: tail_full(bb)))(b)
            if pending_tail is not None:
                pending_tail()

    nc.compile()
    return nc


_NC = None


def _get_program():
    global _NC
    if _NC is None:
        _NC = _build_program()
    return _NC


def make_in_maps(hidden, encoder_outputs, seq_mask, attn_w, attn_b, score_w):
    """Slice/relayout/quantize the full inputs into 8 per-core input maps."""
    hidden = np.asarray(hidden, dtype=np.float32)
    encoder_outputs = np.asarray(encoder_outputs, dtype=np.float32)
    seq_mask = np.asarray(seq_mask, dtype=np.int32)
    attn_w = np.asarray(attn_w, dtype=np.float32)
    attn_b = np.asarray(attn_b, dtype=np.float32)
    score_w = np.asarray(score_w, dtype=np.float32)

    e4 = ml_dtypes.float8_e4m3fn
    bf = ml_dtypes.bfloat16

    # We^T [hin, ho] scaled into e4m3's normal range; pair layout
    # [KP, P, 2, ho]: hin = kp*256 + i*128 + p
    weT = attn_w[:, H:].T * WSCALE
    weP = np.ascontiguousarray(
        weT.reshape(KP, 2, P, H).transpose(0, 2, 1, 3).reshape(KP, P, 2 * H)
    ).astype(e4)

    # enc pair layout per batch: [B, KP, P, 2, S]
    encT = encoder_outputs.transpose(1, 2, 0)  # [B, H, S]
    encPf = encT.reshape(B, KP, 2, P, S).transpose(0, 1, 3, 2, 4).reshape(
        B, KP, P, 2 * S
    )
    encPq = np.ascontiguousarray(encPf).astype(e4)

    # bias columns: Wh @ hidden_b + attn_b, exact f32, [P, B*HB] with
    # col = b*HB + hb, row p = h_out hb*128+p
    hidb = hidden[0] @ attn_w[:, :H].T + attn_b  # [B, H]
    biasc_all = hidb.reshape(B, HB, P).transpose(2, 0, 1).reshape(P, B * HB)
    biasc_all = np.ascontiguousarray(biasc_all.astype(np.float32))

    scorec = np.ascontiguousarray(score_w[0].reshape(HB, P).T).astype(bf)

    # additive mask offsets: 0 keep, -50 masked (exp -> ~1e-22)
    moff_all = np.where(seq_mask != 0, np.float32(-50.0), np.float32(0.0))

    in_maps = []
    for c in range(NCORES):
        bsl = slice(c * BL, (c + 1) * BL)
        in_maps.append(
            {
                "encP": np.ascontiguousarray(encPq[bsl]),
                "weP": weP,
                "biasc": np.ascontiguousarray(
                    biasc_all[:, c * BL * HB : (c + 1) * BL * HB]
                ),
                "scorec": scorec,
                "moff": np.ascontiguousarray(moff_all[bsl]),
            }
        )
    return in_maps


def gather_output(results):
    outs = np.concatenate([results[c]["out"] for c in range(NCORES)], axis=0)
    return np.ascontiguousarray(outs[:, None, :].astype(np.float32))


def kernel(hidden, encoder_outputs, seq_mask, attn_w, attn_b, score_w):
    nc = _get_program()
    in_maps = make_in_maps(
        hidden, encoder_outputs, seq_mask, attn_w, attn_b, score_w
    )
    last_err = None
    for _attempt in range(3):
        try:
            res = run_bass_kernel_spmd(nc, in_maps, list(range(NCORES)))
            return gather_output(res.results)
        except Exception as e:  # rare transient NRT device errors on first exec
            last_err = e
            import time as _time

            _time.sleep(2.0)
    raise last_err
